# revision 22
# baseline (speedup 1.0000x reference)
"""DeepBasisKernel on 8 TRN2 NeuronCores.

K[b] = sum_n softplus(w)[n] * sum_k fx[n,b,k]*fy[n,b,k], where fx/fy are
32 tiny per-basis MLPs (3 -> 5 -> 5 -> 5 -> 16, softplus x3, sigmoid*2-1)
applied to x and y.

Strategy (data-parallel over batch, 8 cores):
 - batch on the free axis, the 64 tiny nets (32 x-nets + 32 y-nets) packed
   block-diagonally along partitions in 3 partition-tiles (24/24/16 nets).
 - Each layer = block-diagonal fp32r matmul (bias folded in via a constant
   ones-row that self-propagates through the layers).
 - softplus = Exp pass + Ln(x+1) pass on ACT (no native Softplus on this
   toolchain); final sigmoid*2-1 = tanh(0.5*z) in one ACT pass over a
   paired [FX | FY] psum tile.
 - fx*fy product on DVE; weighted partition-reduce (wp folded into lhsT)
   via PE matmuls accumulating into a [1, 512] psum tile; DMA straight out.
"""

import sys

if "/opt/trn_rl_repo" not in sys.path:
    sys.path.insert(0, "/opt/trn_rl_repo")

import numpy as np

import bass_rust as _bass_rust
import concourse.bacc as bacc
import concourse.mybir as mybir
from concourse.hw_specs import get_activation_tables
from concourse.tile import TileContext
from concourse.tile_rust import add_dep_helper
from concourse import bass_utils


class _Bacc(bacc.Bacc):
    """Bacc with a steered ACT-table chooser: the greedy chooser picks the
    first set containing each function, so Ln would land in 'natural_log'
    (no Exp) and every Exp<->Ln transition would reload the table (1283ns
    each). Masking 'natural_log' makes Ln choose
    'natural_log_exp_and_others', which also serves Exp; Tanh then lives in
    'exp_and_others' which also serves Exp. Steady state: 2 loads per block
    instead of ~18."""

    def insert_act_table_loads(self):
        has_activation = any(
            isinstance(i, mybir.InstActivation)
            for b in self.main_func.blocks
            for i in b.instructions
        )
        if not has_activation:
            return
        tables = []
        for name, s in get_activation_tables(self.m.arch).items():
            if name == "natural_log":
                s = set()
            tables.append((name, s))
        _bass_rust.insert_act_table_loads(self, tables)

N_BASIS = 32
DATA_DIM = 3
BASIS_DIM = 16
WIDTH = 5
BATCH = 262144
N_CORES = 8
B_C = BATCH // N_CORES  # 32768 per core

# net packing: net ids 0..63 (0..31 = x-nets, 32..63 = y-nets)
PT_BASE = [0, 24, 48]          # first net id of each partition-tile
PT_NETS = [24, 24, 16]         # nets per partition-tile
PT_ROWS = [120, 120, 80]       # hidden rows per tile (5 per net)
# output groups of 8 nets -> 128 psum rows (16 k-outputs per net)
GRP_TILE = [0, 0, 0, 1, 1, 1, 2, 2]   # owning partition-tile of group g
C1 = float(np.log(np.e - 1.0))  # softplus(C1) == 1 exactly: ones-row propagator

W_BLK = 2048       # batch columns per pipeline block
MM_N = 512         # matmul free-dim (one fp32 psum bank)

F32 = mybir.dt.float32
F32R = mybir.dt.float32r
AFT = mybir.ActivationFunctionType


def _ptile_of_net(n):
    for t in range(3):
        if PT_BASE[t] <= n < PT_BASE[t] + PT_NETS[t]:
            return t, n - PT_BASE[t]
    raise ValueError(n)


def _pack_weights(Wx, bx, Wy, by, w):
    """Pack all layer weights into one [128, NCOL] fp32 array (lhsT layouts),
    plus return the column offsets of each block."""
    Wx1, Wx2, Wx3, Wx4 = Wx
    bx1, bx2, bx3, bx4 = bx
    Wy1, Wy2, Wy3, Wy4 = Wy
    by1, by2, by3, by4 = by

    def net_params(n):
        if n < N_BASIS:
            i = n
            return ((Wx1[i], bx1[i]), (Wx2[i], bx2[i]), (Wx3[i], bx3[i]),
                    (Wx4[i], bx4[i]))
        i = n - N_BASIS
        return ((Wy1[i], by1[i]), (Wy2[i], by2[i]), (Wy3[i], by3[i]),
                (Wy4[i], by4[i]))

    cols = {}
    blocks = []
    ncol = 0

    def add(name, arr):
        nonlocal ncol
        cols[name] = ncol
        blocks.append((ncol, arr))
        ncol += arr.shape[1]

    # L1 lhsT: [7, rows_t + 1]
    for t in range(3):
        K = PT_ROWS[t] + 1
        m = np.zeros((7, K), np.float32)
        for p in range(PT_NETS[t]):
            n = PT_BASE[t] + p
            (W1, b1), _, _, _ = net_params(n)
            r0 = 0 if n < N_BASIS else 3
            for wv in range(WIDTH):
                m[r0:r0 + 3, 5 * p + wv] = W1[:, wv]
                m[6, 5 * p + wv] = b1[wv]
        m[6, K - 1] = C1
        add(f"l1_{t}", m)

    # L2/L3 lhsT: [rows_t+1, rows_t+1]
    for li, lname in ((1, "l2"), (2, "l3")):
        for t in range(3):
            K = PT_ROWS[t] + 1
            m = np.zeros((K, K), np.float32)
            for p in range(PT_NETS[t]):
                n = PT_BASE[t] + p
                Wl, bl = net_params(n)[li]
                for v in range(WIDTH):
                    m[5 * p:5 * p + 5, 5 * p + v] = Wl[:, v]
                    m[K - 1, 5 * p + v] = bl[v]
            m[K - 1, K - 1] = C1
            add(f"{lname}_{t}", m)

    # L4 lhsT per group g=0..7: [rows_t+1, 128]
    for g in range(8):
        t = GRP_TILE[g]
        K = PT_ROWS[t] + 1
        m = np.zeros((K, 128), np.float32)
        for ii in range(8):
            n = 8 * g + ii  # net id (g>=4 -> y nets 32..63)
            _, p = _ptile_of_net(n)
            _, _, _, (W4, b4) = net_params(n)
            for k in range(BASIS_DIM):
                m[5 * p:5 * p + 5, 16 * ii + k] = W4[:, k]
                m[K - 1, 16 * ii + k] = b4[k]
        add(f"l4_{g}", m)

    # wp product-scale vectors per x-group j: [128, 1]
    wp = np.logaddexp(0.0, w.astype(np.float64)).astype(np.float32)  # softplus
    for j in range(4):
        m = np.zeros((128, 1), np.float32)
        for ii in range(8):
            m[16 * ii:16 * ii + 16, 0] = wp[8 * j + ii]
        add(f"wp_{j}", m)
    add("ones", np.ones((128, 1), np.float32))

    wtile = np.zeros((128, ncol), np.float32)
    for c0, arr in blocks:
        wtile[:arr.shape[0], c0:c0 + arr.shape[1]] = arr
    return wtile, cols


def build_bass(b_c=B_C, w_blk=W_BLK, wcols=2200):
    """Build the single-core program (SPMD: same program on all cores)."""
    nc = _Bacc("TRN2", target_bir_lowering=False, debug=False)
    xy_d = nc.dram_tensor("xy", [7, b_c], F32R, kind="ExternalInput")
    wt_d = nc.dram_tensor("wt", [128, wcols], F32R, kind="ExternalInput")
    out_d = nc.dram_tensor("out", [1, b_c], F32, kind="ExternalOutput")

    n_blk = b_c // w_blk
    n_sub = w_blk // MM_N

    with TileContext(nc) as tc:
        with (
            tc.tile_pool(name="wpool", bufs=1) as wpool,
            tc.tile_pool(name="xpool", bufs=2) as xpool,
            tc.tile_pool(name="hpool", bufs=1, space="PSUM") as hpool,
            tc.tile_pool(name="fpool", bufs=2, space="PSUM") as fpool,
            tc.tile_pool(name="epool", bufs=3) as epool,
            tc.tile_pool(name="apool", bufs=5) as apool,
            tc.tile_pool(name="spool", bufs=4) as spool,
            tc.tile_pool(name="ppool", bufs=10) as ppool,
        ):
            wt = wpool.tile([128, wcols], F32R)
            nc.sync.dma_start(out=wt, in_=wt_d.ap())

            # column offsets must match _pack_weights
            col = {}
            c = 0
            for t in range(3):
                col[f"l1_{t}"] = c
                c += PT_ROWS[t] + 1
            for lname in ("l2", "l3"):
                for t in range(3):
                    col[f"{lname}_{t}"] = c
                    c += PT_ROWS[t] + 1
            for g in range(8):
                col[f"l4_{g}"] = c
                c += 128
            for j in range(4):
                col[f"wp_{j}"] = c
                c += 1
            col["ones"] = c
            c += 1
            assert c <= wcols

            def wsl(name, k, m):
                c0 = col[name]
                return wt[0:k, c0:c0 + m]

            # chain ACT ops in emission order: keeps all Exp/Ln of a block
            # together, then the block's Tanh ops — minimizes ACT table loads
            prev_act = [None]

            def act(*args, **kwargs):
                inst = nc.scalar.activation(*args, **kwargs).ins
                if prev_act[0] is not None:
                    add_dep_helper(inst, prev_act[0], sync=False,
                                   reason="act table order")
                prev_act[0] = inst
                return inst

            for blk in range(n_blk):
                c0 = blk * w_blk
                xy = xpool.tile([7, w_blk], F32R)
                nc.sync.dma_start(out=xy, in_=xy_d.ap()[:, c0:c0 + w_blk])

                a_prev = [None, None, None]  # rhs tiles per ptile
                for li, lname in enumerate(("l1", "l2", "l3")):
                    a_cur = [None, None, None]
                    for t in range(3):
                        K = PT_ROWS[t] + 1
                        if li == 0:
                            rhs_t, rhs_k = xy, 7
                        else:
                            rhs_t, rhs_k = a_prev[t], K
                        lhsT = wsl(f"{lname}_{t}", rhs_k, K)
                        h = hpool.tile([K, w_blk], F32, tag="h")
                        for s in range(n_sub):
                            sl = slice(s * MM_N, (s + 1) * MM_N)
                            nc.tensor.matmul(
                                h[:, sl], lhsT, rhs_t[0:rhs_k, sl],
                                start=True, stop=True)
                        e = epool.tile([K, w_blk], F32, tag="e")
                        act(e, h, AFT.Exp)
                        # Ln output rounds to fp32r for the next matmul
                        a = apool.tile([K, w_blk], F32R, tag="a")
                        act(a, e, AFT.Ln, bias=1.0)
                        a_cur[t] = a
                    a_prev = a_cur

                # f stage: paired [FX_j | FY_j] over MM_N batch cols
                ko_s = spool.tile([1, w_blk], F32, tag="ko")
                qs = []
                for s in range(n_sub):
                    sl = slice(s * MM_N, (s + 1) * MM_N)
                    ps = []
                    for j in range(4):
                        f = fpool.tile([128, 2 * MM_N], F32, tag="f")
                        for half, g in ((0, j), (1, j + 4)):
                            t = GRP_TILE[g]
                            K = PT_ROWS[t] + 1
                            nc.tensor.matmul(
                                f[:, half * MM_N:(half + 1) * MM_N],
                                wsl(f"l4_{g}", K, 128),
                                a_prev[t][:, sl],
                                start=True, stop=True)
                        fs = spool.tile([128, 2 * MM_N], F32, tag="fs")
                        act(fs, f, AFT.Tanh, scale=0.5)
                        p = ppool.tile([128, MM_N], F32, tag="p")
                        # p = (fx * wp) * fy  -- wp folded into the product
                        wpj = wt[0:128, col[f"wp_{j}"]:col[f"wp_{j}"] + 1].bitcast(F32)
                        nc.vector.scalar_tensor_tensor(
                            p, fs[:, 0:MM_N], wpj, fs[:, MM_N:2 * MM_N],
                            op0=mybir.AluOpType.mult, op1=mybir.AluOpType.mult)
                        ps.append(p)
                    q01 = ppool.tile([128, MM_N], F32, tag="q")
                    nc.gpsimd.tensor_add(q01, ps[0], ps[1])
                    q23 = ppool.tile([128, MM_N], F32, tag="q")
                    nc.gpsimd.tensor_add(q23, ps[2], ps[3])
                    q = ppool.tile([128, MM_N], F32, tag="q")
                    nc.gpsimd.tensor_add(q, q01, q23)
                    qs.append(q)
                # deferred reduce: emitted after the whole f phase so the
                # kout psum tiles (tag 'f') grab slots only when the tanh
                # stream is done -- they fill the next block's hidden phase
                for s, q in enumerate(qs):
                    sl = slice(s * MM_N, (s + 1) * MM_N)
                    kout = fpool.tile([1, MM_N], F32, tag="f")
                    # plain fp32 matmul (slow path, 1 per 512 cols): avoids
                    # fp32r rounding requirements on the DVE/Pool product path
                    nc.tensor.matmul(
                        kout, wsl("ones", 128, 1).bitcast(F32), q,
                        start=True, stop=True)
                    nc.vector.tensor_copy(ko_s[:, sl], kout)
                nc.sync.dma_start(
                    out=out_d.ap()[:, c0:c0 + w_blk], in_=ko_s)

    nc.compile()
    return nc


def _prep_inputs(x, y, Wx1, bx1, Wx2, bx2, Wx3, bx3, Wx4, bx4,
                 Wy1, by1, Wy2, by2, Wy3, by3, Wy4, by4, w):
    wtile, _ = _pack_weights(
        (Wx1, Wx2, Wx3, Wx4), (bx1, bx2, bx3, bx4),
        (Wy1, Wy2, Wy3, Wy4), (by1, by2, by3, by4), w)
    wcols = 2200
    wfull = np.zeros((128, wcols), np.float32)
    wfull[:, :wtile.shape[1]] = wtile

    b = x.shape[0]
    xy = np.empty((7, b), np.float32)
    xy[0:3] = x.T
    xy[3:6] = y.T
    xy[6] = 1.0
    return _round_f32r(xy), _round_f32r(wfull)


def _round_f32r(a):
    # pre-round to fp32r (e8m11): on-chip values == these exactly
    u = np.ascontiguousarray(a, np.float32).view(np.uint32)
    u = (u + np.uint32(0x800)) & np.uint32(0xFFFFF000)
    return u.view(np.float32)


_CACHED = {}


def kernel(**inputs):
    xy, wfull = _prep_inputs(**inputs)
    b = xy.shape[1]
    b_c = b // N_CORES

    key = (b_c,)
    if key not in _CACHED:
        _CACHED[key] = build_bass(b_c=b_c)
    nc = _CACHED[key]

    in_maps = [
        {"xy": np.ascontiguousarray(xy[:, i * b_c:(i + 1) * b_c]),
         "wt": wfull}
        for i in range(N_CORES)
    ]
    res = bass_utils.run_bass_kernel_spmd(nc, in_maps, core_ids=list(range(N_CORES)))
    out = np.concatenate([res.results[i]["out"][0] for i in range(N_CORES)])
    return out.astype(np.float32)
